# revision 9
# baseline (speedup 1.0000x reference)
"""Bass/Trainium2 kernel for nn_BlockSSM (linear block state-space model).

Math (from the reference):
    s_{k+1} = 2*(s_k @ Wx.T + bx + u_k @ Wu.T + bu) + (d_k @ Wd.T + bd)
            = s_k @ A + u_k @ Bu + d_k @ Bd + c
      where A = 2*Wx.T, Bu = 2*Wu.T, Bd = Wd.T, c = 2*bx + 2*bu + bd
    X[k] = s_{k+1}
    Y[k] = s_{k+1} @ Wy.T + by
    reg_error: every stat is multiplied by a 0.0 coefficient (Q_sub's term is
    0.2 * sum([]) == 0), so reg_error == 0.0 exactly.

Strategy: data-parallel over batch (1024 -> 128 per core on 8 cores).
All matmuls run in bf16 with fp32 PSUM accumulation. The state transpose
needed for the next step's matmul (lhsT layout) is done with the DMA xbar
transpose engine, off the compute engines' critical path. The per-step
bias is injected with an all-ones [128,128] lhsT against a bias/128
replicated rhs, so the full state accumulates in PSUM and is evacuated
once (fp32 for the X output via ScalarE, bf16 for the recurrence via
VectorE).
"""

import sys

for _p in ("/opt/trn_rl_repo",):
    if _p not in sys.path:
        sys.path.insert(0, _p)

import numpy as np
import ml_dtypes

import concourse.bacc as bacc
import concourse.mybir as mybir
from concourse.tile import TileContext
from concourse.bass_utils import run_bass_kernel_spmd

BF16 = mybir.dt.bfloat16
F32 = mybir.dt.float32
NPBF16 = ml_dtypes.bfloat16

N_CORES = 8
T, B = 64, 1024
NX, NU, ND, NY = 1024, 512, 256, 256
BL = B // N_CORES  # 128 batch rows per core
KX, KU, KD = NX // 128, NU // 128, ND // 128  # k-tile counts: 8, 4, 2


def build(t_steps: int = T):
    nc = bacc.Bacc("TRN2", target_bir_lowering=False, debug=False,
                   num_devices=N_CORES)

    xT0 = nc.dram_tensor("xT0", [128, NX], BF16, kind="ExternalInput")
    UT = nc.dram_tensor("UT", [t_steps, 128, NU], BF16, kind="ExternalInput")
    DT = nc.dram_tensor("DT", [t_steps, 128, ND], BF16, kind="ExternalInput")
    WA = nc.dram_tensor("WA", [128, KX * NX], BF16, kind="ExternalInput")
    WU = nc.dram_tensor("WU", [128, KU * NX], BF16, kind="ExternalInput")
    WD = nc.dram_tensor("WD", [128, KD * NX], BF16, kind="ExternalInput")
    WY = nc.dram_tensor("WY", [128, KX * NY], BF16, kind="ExternalInput")
    CR = nc.dram_tensor("CR", [128, NX], BF16, kind="ExternalInput")
    BYR = nc.dram_tensor("BYR", [128, NY], BF16, kind="ExternalInput")
    ONE = nc.dram_tensor("ONE", [128, 128], BF16, kind="ExternalInput")
    XO = nc.dram_tensor("XO", [t_steps, 128, NX], F32, kind="ExternalOutput")
    YO = nc.dram_tensor("YO", [t_steps, 128, NY], F32, kind="ExternalOutput")

    with TileContext(nc) as tc:
        with (
            tc.tile_pool(name="wpool", bufs=1) as wpool,
            tc.tile_pool(name="upool", bufs=4) as upool,
            tc.tile_pool(name="dpool", bufs=4) as dpool,
            tc.tile_pool(name="xtpool", bufs=3) as xtpool,
            tc.tile_pool(name="xbfpool", bufs=3) as xbfpool,
            tc.tile_pool(name="xspool", bufs=4) as xspool,
            tc.tile_pool(name="yspool", bufs=4) as yspool,
            tc.tile_pool(name="pxpool", bufs=2, space="PSUM") as pxpool,
            tc.tile_pool(name="pypool", bufs=2, space="PSUM") as pypool,
        ):
            wa = wpool.tile([128, KX * NX], BF16, name="wa")
            nc.gpsimd.dma_start(wa[:], WA[:])
            wu = wpool.tile([128, KU * NX], BF16, name="wu")
            nc.gpsimd.dma_start(wu[:], WU[:])
            wd = wpool.tile([128, KD * NX], BF16, name="wd")
            nc.gpsimd.dma_start(wd[:], WD[:])
            wy = wpool.tile([128, KX * NY], BF16, name="wy")
            nc.gpsimd.dma_start(wy[:], WY[:])
            cr = wpool.tile([128, NX], BF16, name="cr")
            nc.gpsimd.dma_start(cr[:], CR[:])
            byr = wpool.tile([128, NY], BF16, name="byr")
            nc.gpsimd.dma_start(byr[:], BYR[:])
            one = wpool.tile([128, 128], BF16, name="one")
            nc.gpsimd.dma_start(one[:], ONE[:])

            xT_cur = xtpool.tile([128, NX], BF16, name="xT")
            nc.gpsimd.dma_start(xT_cur[:], xT0[:])

            for k in range(t_steps):
                ut = upool.tile([128, NU], BF16, name="ut")
                nc.gpsimd.dma_start(ut[:], UT[k])
                dt = dpool.tile([128, ND], BF16, name="dt")
                nc.gpsimd.dma_start(dt[:], DT[k])

                px_lo = pxpool.tile([128, 512], F32, name="px_lo")
                px_hi = pxpool.tile([128, 512], F32, name="px_hi")
                py = pypool.tile([128, NY], F32, name="py") if k >= 1 else None

                # Y bias via ones-matmul: ones.T @ (by/128 replicated)
                if py is not None:
                    nc.tensor.matmul(py[:], lhsT=one[:], rhs=byr[:],
                                     start=True, stop=False)

                xbf = xbfpool.tile([128, NX], BF16, name="xbf")
                xstage = xspool.tile([128, NX], F32, name="xstage")
                xT_next = xtpool.tile([128, NX], BF16, name="xT")

                # phase 1: everything feeding the LOW half of the state so
                # px_lo finishes early and its transposes start ~3us sooner
                for half, px in ((0, px_lo), (1, px_hi)):
                    off = half * 512
                    for kt in range(KU):
                        nc.tensor.matmul(px[:], lhsT=ut[:, kt * 128:(kt + 1) * 128],
                                         rhs=wu[:, kt * NX + off:kt * NX + off + 512],
                                         start=(kt == 0), stop=False)
                    for kt in range(KD):
                        nc.tensor.matmul(px[:], lhsT=dt[:, kt * 128:(kt + 1) * 128],
                                         rhs=wd[:, kt * NX + off:kt * NX + off + 512],
                                         start=False, stop=False)
                    for i in range(KX):
                        nc.tensor.matmul(px[:], lhsT=xT_cur[:, i * 128:(i + 1) * 128],
                                         rhs=wa[:, i * NX + off:i * NX + off + 512],
                                         start=False, stop=(i == KX - 1))
                    # evacuate this half (c-bias folded in on DVE), then
                    # kick its four transposes on a dedicated HWDGE engine
                    nc.vector.tensor_add(out=xbf[:, off:off + 512], in0=px[:],
                                         in1=cr[:, off:off + 512])
                    eng = nc.sync if half == 0 else nc.scalar
                    for i in range(4 * half, 4 * half + 4):
                        eng.dma_start_transpose(
                            xT_next[:, i * 128:(i + 1) * 128],
                            xbf[:, i * 128:(i + 1) * 128])

                # previous step's output head: Y[k-1] = s_k @ Wy.T + by
                # (same lhsT tiles; fills the PE while transposes run)
                if py is not None:
                    for i in range(KX):
                        lh = xT_cur[:, i * 128:(i + 1) * 128]
                        nc.tensor.matmul(py[:], lhsT=lh,
                                         rhs=wy[:, i * NY:(i + 1) * NY],
                                         start=False, stop=(i == KX - 1))

                # fp32 outputs trail everything latency-critical
                nc.vector.tensor_add(out=xstage[:, 0:512], in0=px_lo[:],
                                     in1=cr[:, 0:512])
                nc.vector.tensor_add(out=xstage[:, 512:1024], in0=px_hi[:],
                                     in1=cr[:, 512:1024])
                nc.gpsimd.dma_start(XO[k], xstage[:])
                if py is not None:
                    ystage = yspool.tile([128, NY], F32, name="ystage")
                    nc.vector.tensor_copy(out=ystage[:], in_=py[:])
                    nc.gpsimd.dma_start(YO[k - 1], ystage[:])

                xT_cur = xT_next

            # epilogue: Y[T-1] = s_T @ Wy.T + by
            py = pypool.tile([128, NY], F32, name="py")
            nc.tensor.matmul(py[:], lhsT=one[:], rhs=byr[:],
                             start=True, stop=False)
            for i in range(KX):
                last = i == KX - 1
                lh = xT_cur[:, i * 128:(i + 1) * 128]
                nc.tensor.matmul(py[:], lhsT=lh,
                                 rhs=wy[:, i * NY:(i + 1) * NY],
                                 start=False, stop=last)
            ystage = yspool.tile([128, NY], F32, name="ystage")
            nc.scalar.copy(out=ystage[:], in_=py[:])
            nc.gpsimd.dma_start(YO[t_steps - 1], ystage[:])

    nc.compile()
    return nc


def _to_lhsT_tiles(mat: np.ndarray) -> np.ndarray:
    """[rows, cols] -> [128, (rows//128)*cols] laid out so that
    out[p, i*cols + j] = mat[i*128 + p, j] (k-tile-major free dim)."""
    rows, cols = mat.shape
    kt = rows // 128
    return (mat.reshape(kt, 128, cols).transpose(1, 0, 2)
            .reshape(128, kt * cols))


def _prep_weights(Wx, bx, Wu, bu, Wd, bd, Wy, by):
    A = (2.0 * Wx.T).astype(np.float32)          # [NX, NX]
    Bu = (2.0 * Wu.T).astype(np.float32)         # [NU, NX]
    Bd = Wd.T.astype(np.float32)                 # [ND, NX]
    Cy = Wy.T.astype(np.float32)                 # [NX, NY]
    c = (2.0 * bx + 2.0 * bu + bd).astype(np.float32)

    wa = _to_lhsT_tiles(A).astype(NPBF16)
    wu = _to_lhsT_tiles(Bu).astype(NPBF16)
    wd = _to_lhsT_tiles(Bd).astype(NPBF16)
    wy = _to_lhsT_tiles(Cy).astype(NPBF16)
    cr = np.broadcast_to(c.astype(NPBF16), (128, NX)).copy()
    byr = np.broadcast_to((by.astype(np.float32) / 128.0).astype(NPBF16),
                          (128, NY)).copy()
    one = np.ones((128, 128), dtype=NPBF16)
    return dict(WA=wa, WU=wu, WD=wd, WY=wy, CR=cr, BYR=byr, ONE=one)


def _prep_seq(M: np.ndarray) -> np.ndarray:
    """[t, BL, F] fp32 -> [t, 128, F] bf16 transposed per step:
    out[t, p, kt*128+m] = M[t, m, kt*128+p]."""
    t, bl, f = M.shape
    kt = f // 128
    out = (M.transpose(0, 2, 1)                 # [t, F, BL]
           .reshape(t, kt, 128, bl)             # [t, kt, p, m]
           .transpose(0, 2, 1, 3)               # [t, p, kt, m]
           .reshape(t, 128, kt * bl))
    return np.ascontiguousarray(out).astype(NPBF16)


_NC_CACHE = {}


def _get_nc(t_steps: int = T):
    if t_steps not in _NC_CACHE:
        _NC_CACHE[t_steps] = build(t_steps)
    return _NC_CACHE[t_steps]


def kernel(x, U, D, Wx, bx, Wu, bu, Wd, bd, Wy, by, **run_kwargs):
    x = np.asarray(x, dtype=np.float32)
    U = np.asarray(U, dtype=np.float32)
    D = np.asarray(D, dtype=np.float32)
    Wx = np.asarray(Wx, dtype=np.float32)
    bx = np.asarray(bx, dtype=np.float32)
    Wu = np.asarray(Wu, dtype=np.float32)
    bu = np.asarray(bu, dtype=np.float32)
    Wd = np.asarray(Wd, dtype=np.float32)
    bd = np.asarray(bd, dtype=np.float32)
    Wy = np.asarray(Wy, dtype=np.float32)
    by = np.asarray(by, dtype=np.float32)

    t_steps = U.shape[0]
    nc = _get_nc(t_steps)

    weights = _prep_weights(Wx, bx, Wu, bu, Wd, bd, Wy, by)

    in_maps = []
    for cix in range(N_CORES):
        sl = slice(cix * BL, (cix + 1) * BL)
        xT0 = _to_lhsT_tiles(x[sl].T.copy()).astype(NPBF16)
        in_maps.append({
            "xT0": xT0,
            "UT": _prep_seq(U[:, sl]),
            "DT": _prep_seq(D[:, sl]),
            **weights,
        })

    res = run_bass_kernel_spmd(nc, in_maps, core_ids=list(range(N_CORES)),
                               **run_kwargs)

    X = np.concatenate([res.results[cix]["XO"] for cix in range(N_CORES)],
                       axis=1)
    Y = np.concatenate([res.results[cix]["YO"] for cix in range(N_CORES)],
                       axis=1)
    reg = np.zeros((), dtype=np.float32)
    if run_kwargs:
        kernel.last_results = res
    return X, Y, reg


# revision 15
# speedup vs baseline: 1.5356x; 1.5356x over previous
"""Bass/Trainium2 kernel for nn_BlockSSM (linear block state-space model).

Math (from the reference):
    s_{k+1} = 2*(s_k @ Wx.T + bx + u_k @ Wu.T + bu) + (d_k @ Wd.T + bd)
            = s_k @ A + u_k @ Bu + d_k @ Bd + c
      where A = 2*Wx.T, Bu = 2*Wu.T, Bd = Wd.T, c = 2*bx + 2*bu + bd
    X[k] = s_{k+1}
    Y[k] = s_{k+1} @ Wy.T + by
    reg_error: every stat is multiplied by a 0.0 coefficient (Q_sub's term is
    0.2 * sum([]) == 0), so reg_error == 0.0 exactly.

Strategy: data-parallel over batch (1024 -> 128 per core on 8 cores).
All matmuls run in bf16 with fp32 PSUM accumulation. The state transpose
needed for the next step's matmul (lhsT layout) is done with the DMA xbar
transpose engine, off the compute engines' critical path. The per-step
bias is injected with an all-ones [128,128] lhsT against a bias/128
replicated rhs, so the full state accumulates in PSUM and is evacuated
once (fp32 for the X output via ScalarE, bf16 for the recurrence via
VectorE).
"""

import sys

for _p in ("/opt/trn_rl_repo",):
    if _p not in sys.path:
        sys.path.insert(0, _p)

import numpy as np
import ml_dtypes

import concourse.bacc as bacc
import concourse.mybir as mybir
from concourse.tile import TileContext
from concourse.bass_utils import run_bass_kernel_spmd

BF16 = mybir.dt.bfloat16
F32 = mybir.dt.float32
NPBF16 = ml_dtypes.bfloat16

N_CORES = 8
T, B = 64, 1024
NX, NU, ND, NY = 1024, 512, 256, 256
BL = B // N_CORES  # 128 batch rows per core
KX, KU, KD = NX // 128, NU // 128, ND // 128  # k-tile counts: 8, 4, 2


def build(t_steps: int = T):
    nc = bacc.Bacc("TRN2", target_bir_lowering=False, debug=False,
                   num_devices=N_CORES)

    xT0 = nc.dram_tensor("xT0", [128, NX], BF16, kind="ExternalInput")
    UT = nc.dram_tensor("UT", [t_steps, 128, NU], BF16, kind="ExternalInput")
    DT = nc.dram_tensor("DT", [t_steps, 128, ND], BF16, kind="ExternalInput")
    WA = nc.dram_tensor("WA", [128, KX * NX], BF16, kind="ExternalInput")
    WU = nc.dram_tensor("WU", [128, KU * NX], BF16, kind="ExternalInput")
    WD = nc.dram_tensor("WD", [128, KD * NX], BF16, kind="ExternalInput")
    WY = nc.dram_tensor("WY", [128, KX * NY], BF16, kind="ExternalInput")
    CR = nc.dram_tensor("CR", [128, NX], BF16, kind="ExternalInput")
    BYR = nc.dram_tensor("BYR", [128, NY], BF16, kind="ExternalInput")
    ONE = nc.dram_tensor("ONE", [128, 128], BF16, kind="ExternalInput")
    IDN = nc.dram_tensor("IDN", [128, 128], BF16, kind="ExternalInput")
    XO = nc.dram_tensor("XO", [t_steps, 128, NX], F32, kind="ExternalOutput")
    YO = nc.dram_tensor("YO", [t_steps, 128, NY], F32, kind="ExternalOutput")

    with TileContext(nc) as tc:
        with (
            tc.tile_pool(name="wpool", bufs=1) as wpool,
            tc.tile_pool(name="upool", bufs=4) as upool,
            tc.tile_pool(name="dpool", bufs=4) as dpool,
            tc.tile_pool(name="xtpool", bufs=3) as xtpool,
            tc.tile_pool(name="xbfpool", bufs=3) as xbfpool,
            tc.tile_pool(name="xspool", bufs=4) as xspool,
            tc.tile_pool(name="yspool", bufs=4) as yspool,
            tc.tile_pool(name="pxpool", bufs=2, space="PSUM") as pxpool,
            tc.tile_pool(name="pypool", bufs=2, space="PSUM") as pypool,
            tc.tile_pool(name="ptpool", bufs=2, space="PSUM") as ptpool,
        ):
            wa = wpool.tile([128, KX * NX], BF16, name="wa")
            nc.gpsimd.dma_start(wa[:], WA[:])
            wu = wpool.tile([128, KU * NX], BF16, name="wu")
            nc.gpsimd.dma_start(wu[:], WU[:])
            wd = wpool.tile([128, KD * NX], BF16, name="wd")
            nc.gpsimd.dma_start(wd[:], WD[:])
            wy = wpool.tile([128, KX * NY], BF16, name="wy")
            nc.gpsimd.dma_start(wy[:], WY[:])
            cr = wpool.tile([128, NX], BF16, name="cr")
            nc.gpsimd.dma_start(cr[:], CR[:])
            byr = wpool.tile([128, NY], BF16, name="byr")
            nc.gpsimd.dma_start(byr[:], BYR[:])
            one = wpool.tile([128, 128], BF16, name="one")
            nc.gpsimd.dma_start(one[:], ONE[:])
            idn = wpool.tile([128, 128], BF16, name="idn")
            nc.gpsimd.dma_start(idn[:], IDN[:])

            xT_cur = xtpool.tile([128, NX], BF16, name="xT")
            nc.gpsimd.dma_start(xT_cur[:], xT0[:])

            for k in range(t_steps):
                ut = upool.tile([128, NU], BF16, name="ut")
                nc.gpsimd.dma_start(ut[:], UT[k])
                dt = dpool.tile([128, ND], BF16, name="dt")
                nc.gpsimd.dma_start(dt[:], DT[k])

                px_lo = pxpool.tile([128, 512], F32, name="px_lo")
                px_hi = pxpool.tile([128, 512], F32, name="px_hi")
                py = pypool.tile([128, NY], F32, name="py") if k >= 1 else None

                # Y bias via ones-matmul: ones.T @ (by/128 replicated)
                if py is not None:
                    nc.tensor.matmul(py[:], lhsT=one[:], rhs=byr[:],
                                     start=True, stop=False)

                xbf = xbfpool.tile([128, NX], BF16, name="xbf")
                xstage = xspool.tile([128, NX], F32, name="xstage")
                xT_next = xtpool.tile([128, NX], BF16, name="xT")

                # phase 1: everything feeding the LOW half of the state so
                # px_lo finishes early and its transposes start ~3us sooner
                for half, px in ((0, px_lo), (1, px_hi)):
                    off = half * 512
                    for kt in range(KU):
                        nc.tensor.matmul(px[:], lhsT=ut[:, kt * 128:(kt + 1) * 128],
                                         rhs=wu[:, kt * NX + off:kt * NX + off + 512],
                                         start=(kt == 0), stop=False)
                    for kt in range(KD):
                        nc.tensor.matmul(px[:], lhsT=dt[:, kt * 128:(kt + 1) * 128],
                                         rhs=wd[:, kt * NX + off:kt * NX + off + 512],
                                         start=False, stop=False)
                    for i in range(KX):
                        nc.tensor.matmul(px[:], lhsT=xT_cur[:, i * 128:(i + 1) * 128],
                                         rhs=wa[:, i * NX + off:i * NX + off + 512],
                                         start=False, stop=(i == KX - 1))
                    # evacuate this half with the c-bias folded in on DVE
                    nc.vector.tensor_add(out=xbf[:, off:off + 512], in0=px[:],
                                         in1=cr[:, off:off + 512])
                    if half == 0:
                        # lo tiles: DMA xbar transposes, one engine only —
                        # concurrent xbar streams on two rings corrupt data
                        for i in range(4):
                            nc.sync.dma_start_transpose(
                                xT_next[:, i * 128:(i + 1) * 128],
                                xbf[:, i * 128:(i + 1) * 128])

                # hi tiles: PE transposes (produced last; keeps the DMA xbar
                # pipe short), evacuated from PSUM by DVE
                ptp = ptpool.tile([128, 512], BF16, name="ptp")
                for j in range(4):
                    i = 4 + j
                    nc.tensor.transpose(ptp[:, j * 128:(j + 1) * 128],
                                        xbf[:, i * 128:(i + 1) * 128], idn[:])
                for j in range(4):
                    i = 4 + j
                    nc.vector.tensor_copy(out=xT_next[:, i * 128:(i + 1) * 128],
                                          in_=ptp[:, j * 128:(j + 1) * 128])

                # previous step's output head: Y[k-1] = s_k @ Wy.T + by
                # (same lhsT tiles; fills the PE while transposes run)
                if py is not None:
                    for i in range(KX):
                        lh = xT_cur[:, i * 128:(i + 1) * 128]
                        nc.tensor.matmul(py[:], lhsT=lh,
                                         rhs=wy[:, i * NY:(i + 1) * NY],
                                         start=False, stop=(i == KX - 1))

                # fp32 outputs trail everything latency-critical
                nc.vector.tensor_add(out=xstage[:, 0:512], in0=px_lo[:],
                                     in1=cr[:, 0:512])
                nc.vector.tensor_add(out=xstage[:, 512:1024], in0=px_hi[:],
                                     in1=cr[:, 512:1024])
                nc.gpsimd.dma_start(XO[k], xstage[:])
                if py is not None:
                    ystage = yspool.tile([128, NY], F32, name="ystage")
                    nc.vector.tensor_copy(out=ystage[:], in_=py[:])
                    nc.gpsimd.dma_start(YO[k - 1], ystage[:])

                xT_cur = xT_next

            # epilogue: Y[T-1] = s_T @ Wy.T + by
            py = pypool.tile([128, NY], F32, name="py")
            nc.tensor.matmul(py[:], lhsT=one[:], rhs=byr[:],
                             start=True, stop=False)
            for i in range(KX):
                last = i == KX - 1
                lh = xT_cur[:, i * 128:(i + 1) * 128]
                nc.tensor.matmul(py[:], lhsT=lh,
                                 rhs=wy[:, i * NY:(i + 1) * NY],
                                 start=False, stop=last)
            ystage = yspool.tile([128, NY], F32, name="ystage")
            nc.scalar.copy(out=ystage[:], in_=py[:])
            nc.gpsimd.dma_start(YO[t_steps - 1], ystage[:])

    nc.compile()
    return nc


def _to_lhsT_tiles(mat: np.ndarray) -> np.ndarray:
    """[rows, cols] -> [128, (rows//128)*cols] laid out so that
    out[p, i*cols + j] = mat[i*128 + p, j] (k-tile-major free dim)."""
    rows, cols = mat.shape
    kt = rows // 128
    return (mat.reshape(kt, 128, cols).transpose(1, 0, 2)
            .reshape(128, kt * cols))


def _prep_weights(Wx, bx, Wu, bu, Wd, bd, Wy, by):
    A = (2.0 * Wx.T).astype(np.float32)          # [NX, NX]
    Bu = (2.0 * Wu.T).astype(np.float32)         # [NU, NX]
    Bd = Wd.T.astype(np.float32)                 # [ND, NX]
    Cy = Wy.T.astype(np.float32)                 # [NX, NY]
    c = (2.0 * bx + 2.0 * bu + bd).astype(np.float32)

    wa = _to_lhsT_tiles(A).astype(NPBF16)
    wu = _to_lhsT_tiles(Bu).astype(NPBF16)
    wd = _to_lhsT_tiles(Bd).astype(NPBF16)
    wy = _to_lhsT_tiles(Cy).astype(NPBF16)
    cr = np.broadcast_to(c.astype(NPBF16), (128, NX)).copy()
    byr = np.broadcast_to((by.astype(np.float32) / 128.0).astype(NPBF16),
                          (128, NY)).copy()
    one = np.ones((128, 128), dtype=NPBF16)
    idn = np.eye(128, dtype=NPBF16)
    return dict(WA=wa, WU=wu, WD=wd, WY=wy, CR=cr, BYR=byr, ONE=one, IDN=idn)


def _prep_seq(M: np.ndarray) -> np.ndarray:
    """[t, BL, F] fp32 -> [t, 128, F] bf16 transposed per step:
    out[t, p, kt*128+m] = M[t, m, kt*128+p]."""
    t, bl, f = M.shape
    kt = f // 128
    out = (M.transpose(0, 2, 1)                 # [t, F, BL]
           .reshape(t, kt, 128, bl)             # [t, kt, p, m]
           .transpose(0, 2, 1, 3)               # [t, p, kt, m]
           .reshape(t, 128, kt * bl))
    return np.ascontiguousarray(out).astype(NPBF16)


_NC_CACHE = {}


def _get_nc(t_steps: int = T):
    if t_steps not in _NC_CACHE:
        _NC_CACHE[t_steps] = build(t_steps)
    return _NC_CACHE[t_steps]


def kernel(x, U, D, Wx, bx, Wu, bu, Wd, bd, Wy, by, **run_kwargs):
    x = np.asarray(x, dtype=np.float32)
    U = np.asarray(U, dtype=np.float32)
    D = np.asarray(D, dtype=np.float32)
    Wx = np.asarray(Wx, dtype=np.float32)
    bx = np.asarray(bx, dtype=np.float32)
    Wu = np.asarray(Wu, dtype=np.float32)
    bu = np.asarray(bu, dtype=np.float32)
    Wd = np.asarray(Wd, dtype=np.float32)
    bd = np.asarray(bd, dtype=np.float32)
    Wy = np.asarray(Wy, dtype=np.float32)
    by = np.asarray(by, dtype=np.float32)

    t_steps = U.shape[0]
    nc = _get_nc(t_steps)

    weights = _prep_weights(Wx, bx, Wu, bu, Wd, bd, Wy, by)

    in_maps = []
    for cix in range(N_CORES):
        sl = slice(cix * BL, (cix + 1) * BL)
        xT0 = _to_lhsT_tiles(x[sl].T.copy()).astype(NPBF16)
        in_maps.append({
            "xT0": xT0,
            "UT": _prep_seq(U[:, sl]),
            "DT": _prep_seq(D[:, sl]),
            **weights,
        })

    res = run_bass_kernel_spmd(nc, in_maps, core_ids=list(range(N_CORES)),
                               **run_kwargs)

    X = np.concatenate([res.results[cix]["XO"] for cix in range(N_CORES)],
                       axis=1)
    Y = np.concatenate([res.results[cix]["YO"] for cix in range(N_CORES)],
                       axis=1)
    reg = np.zeros((), dtype=np.float32)
    if run_kwargs:
        kernel.last_results = res
    return X, Y, reg


# revision 17
# speedup vs baseline: 3.0715x; 2.0001x over previous
"""Bass/Trainium2 kernel for nn_BlockSSM (linear block state-space model).

Math (from the reference):
    s_{k+1} = 2*(s_k @ Wx.T + bx + u_k @ Wu.T + bu) + (d_k @ Wd.T + bd)
            = s_k @ A + u_k @ Bu + d_k @ Bd + c
      where A = 2*Wx.T, Bu = 2*Wu.T, Bd = Wd.T, c = 2*bx + 2*bu + bd
    X[k] = s_{k+1}
    Y[k] = s_{k+1} @ Wy.T + by
    reg_error: every stat is multiplied by a 0.0 coefficient (Q_sub's term is
    0.2 * sum([]) == 0), so reg_error == 0.0 exactly.

Strategy: data-parallel over batch (1024 -> 128 per core on 8 cores).
All matmuls run in bf16 with fp32 PSUM accumulation. The state transpose
needed for the next step's matmul (lhsT layout) is done with the DMA xbar
transpose engine, off the compute engines' critical path. The per-step
bias is injected with an all-ones [128,128] lhsT against a bias/128
replicated rhs, so the full state accumulates in PSUM and is evacuated
once (fp32 for the X output via ScalarE, bf16 for the recurrence via
VectorE).
"""

import sys

for _p in ("/opt/trn_rl_repo",):
    if _p not in sys.path:
        sys.path.insert(0, _p)

import numpy as np
import ml_dtypes

import concourse.bacc as bacc
import concourse.mybir as mybir
from concourse.tile import TileContext
from concourse.bass_utils import run_bass_kernel_spmd

BF16 = mybir.dt.bfloat16
F32 = mybir.dt.float32
NPBF16 = ml_dtypes.bfloat16

N_CORES = 8
T, B = 64, 1024
NX, NU, ND, NY = 1024, 512, 256, 256
BL = B // N_CORES  # 128 batch rows per core
KX, KU, KD = NX // 128, NU // 128, ND // 128  # k-tile counts: 8, 4, 2


def build(t_steps: int = T):
    nc = bacc.Bacc("TRN2", target_bir_lowering=False, debug=False,
                   num_devices=N_CORES)

    xT0 = nc.dram_tensor("xT0", [128, NX], BF16, kind="ExternalInput")
    UT = nc.dram_tensor("UT", [t_steps, 128, NU], BF16, kind="ExternalInput")
    DT = nc.dram_tensor("DT", [t_steps, 128, ND], BF16, kind="ExternalInput")
    WA = nc.dram_tensor("WA", [128, KX * NX], BF16, kind="ExternalInput")
    WU = nc.dram_tensor("WU", [128, KU * NX], BF16, kind="ExternalInput")
    WD = nc.dram_tensor("WD", [128, KD * NX], BF16, kind="ExternalInput")
    WY = nc.dram_tensor("WY", [128, KX * NY], BF16, kind="ExternalInput")
    CR = nc.dram_tensor("CR", [128, NX], BF16, kind="ExternalInput")
    BYR = nc.dram_tensor("BYR", [128, NY], BF16, kind="ExternalInput")
    ONE = nc.dram_tensor("ONE", [128, 128], BF16, kind="ExternalInput")
    IDN = nc.dram_tensor("IDN", [128, 128], BF16, kind="ExternalInput")
    XO = nc.dram_tensor("XO", [t_steps, 128, NX], F32, kind="ExternalOutput")
    YO = nc.dram_tensor("YO", [t_steps, 128, NY], F32, kind="ExternalOutput")

    with TileContext(nc) as tc:
        with (
            tc.tile_pool(name="wpool", bufs=1) as wpool,
            tc.tile_pool(name="upool", bufs=4) as upool,
            tc.tile_pool(name="dpool", bufs=4) as dpool,
            tc.tile_pool(name="xtpool", bufs=3) as xtpool,
            tc.tile_pool(name="xbfpool", bufs=3) as xbfpool,
            tc.tile_pool(name="xspool", bufs=4) as xspool,
            tc.tile_pool(name="yspool", bufs=4) as yspool,
            tc.tile_pool(name="pxpool", bufs=2, space="PSUM") as pxpool,
            tc.tile_pool(name="pypool", bufs=2, space="PSUM") as pypool,
            tc.tile_pool(name="ptpool", bufs=1, space="PSUM") as ptpool,
        ):
            wa = wpool.tile([128, KX * NX], BF16, name="wa")
            nc.gpsimd.dma_start(wa[:], WA[:])
            wu = wpool.tile([128, KU * NX], BF16, name="wu")
            nc.gpsimd.dma_start(wu[:], WU[:])
            wd = wpool.tile([128, KD * NX], BF16, name="wd")
            nc.gpsimd.dma_start(wd[:], WD[:])
            wy = wpool.tile([128, KX * NY], BF16, name="wy")
            nc.gpsimd.dma_start(wy[:], WY[:])
            cr = wpool.tile([128, NX], BF16, name="cr")
            nc.gpsimd.dma_start(cr[:], CR[:])
            byr = wpool.tile([128, NY], BF16, name="byr")
            nc.gpsimd.dma_start(byr[:], BYR[:])
            one = wpool.tile([128, 128], BF16, name="one")
            nc.gpsimd.dma_start(one[:], ONE[:])
            idn = wpool.tile([128, 128], BF16, name="idn")
            nc.gpsimd.dma_start(idn[:], IDN[:])

            xT_cur = xtpool.tile([128, NX], BF16, name="xT")
            nc.gpsimd.dma_start(xT_cur[:], xT0[:])

            for k in range(t_steps):
                ut = upool.tile([128, NU], BF16, name="ut")
                nc.gpsimd.dma_start(ut[:], UT[k])
                dt = dpool.tile([128, ND], BF16, name="dt")
                nc.gpsimd.dma_start(dt[:], DT[k])

                px_lo = pxpool.tile([128, 512], F32, name="px_lo")
                px_hi = pxpool.tile([128, 512], F32, name="px_hi")
                py = pypool.tile([128, NY], F32, name="py") if k >= 1 else None

                # Y bias via ones-matmul: ones.T @ (by/128 replicated)
                if py is not None:
                    nc.tensor.matmul(py[:], lhsT=one[:], rhs=byr[:],
                                     start=True, stop=False)

                xbf = xbfpool.tile([128, NX], BF16, name="xbf")
                xstage = xspool.tile([128, NX], F32, name="xstage")
                xT_next = xtpool.tile([128, NX], BF16, name="xT")

                # phase 1: everything feeding the LOW half of the state so
                # px_lo finishes early and its transposes start ~3us sooner
                for half, px in ((0, px_lo), (1, px_hi)):
                    off = half * 512
                    for kt in range(KU):
                        nc.tensor.matmul(px[:], lhsT=ut[:, kt * 128:(kt + 1) * 128],
                                         rhs=wu[:, kt * NX + off:kt * NX + off + 512],
                                         start=(kt == 0), stop=False)
                    for kt in range(KD):
                        nc.tensor.matmul(px[:], lhsT=dt[:, kt * 128:(kt + 1) * 128],
                                         rhs=wd[:, kt * NX + off:kt * NX + off + 512],
                                         start=False, stop=False)
                    for i in range(KX):
                        nc.tensor.matmul(px[:], lhsT=xT_cur[:, i * 128:(i + 1) * 128],
                                         rhs=wa[:, i * NX + off:i * NX + off + 512],
                                         start=False, stop=(i == KX - 1))
                    # evacuate this half with the c-bias folded in on DVE
                    nc.vector.tensor_add(out=xbf[:, off:off + 512], in0=px[:],
                                         in1=cr[:, off:off + 512])

                # all 8 transposes on the PE (219ns each measured; DMA xbar
                # transposes serialize against all other DMA traffic and
                # corrupt when two streams run concurrently)
                ptp_lo = ptpool.tile([128, 512], BF16, name="ptp_lo")
                ptp_hi = ptpool.tile([128, 512], BF16, name="ptp_hi")
                for i in range(KX):
                    ptp = ptp_lo if i < 4 else ptp_hi
                    j = i % 4
                    nc.tensor.transpose(ptp[:, j * 128:(j + 1) * 128],
                                        xbf[:, i * 128:(i + 1) * 128], idn[:])
                for i in range(KX):
                    ptp = ptp_lo if i < 4 else ptp_hi
                    j = i % 4
                    nc.vector.tensor_copy(out=xT_next[:, i * 128:(i + 1) * 128],
                                          in_=ptp[:, j * 128:(j + 1) * 128])

                # previous step's output head: Y[k-1] = s_k @ Wy.T + by
                # (same lhsT tiles; fills the PE while transposes run)
                if py is not None:
                    for i in range(KX):
                        lh = xT_cur[:, i * 128:(i + 1) * 128]
                        nc.tensor.matmul(py[:], lhsT=lh,
                                         rhs=wy[:, i * NY:(i + 1) * NY],
                                         start=False, stop=(i == KX - 1))

                # fp32 outputs trail everything latency-critical
                nc.vector.tensor_add(out=xstage[:, 0:512], in0=px_lo[:],
                                     in1=cr[:, 0:512])
                nc.vector.tensor_add(out=xstage[:, 512:1024], in0=px_hi[:],
                                     in1=cr[:, 512:1024])
                nc.gpsimd.dma_start(XO[k], xstage[:])
                if py is not None:
                    ystage = yspool.tile([128, NY], F32, name="ystage")
                    nc.vector.tensor_copy(out=ystage[:], in_=py[:])
                    nc.gpsimd.dma_start(YO[k - 1], ystage[:])

                xT_cur = xT_next

            # epilogue: Y[T-1] = s_T @ Wy.T + by
            py = pypool.tile([128, NY], F32, name="py")
            nc.tensor.matmul(py[:], lhsT=one[:], rhs=byr[:],
                             start=True, stop=False)
            for i in range(KX):
                last = i == KX - 1
                lh = xT_cur[:, i * 128:(i + 1) * 128]
                nc.tensor.matmul(py[:], lhsT=lh,
                                 rhs=wy[:, i * NY:(i + 1) * NY],
                                 start=False, stop=last)
            ystage = yspool.tile([128, NY], F32, name="ystage")
            nc.scalar.copy(out=ystage[:], in_=py[:])
            nc.gpsimd.dma_start(YO[t_steps - 1], ystage[:])

    nc.compile()
    return nc


def _to_lhsT_tiles(mat: np.ndarray) -> np.ndarray:
    """[rows, cols] -> [128, (rows//128)*cols] laid out so that
    out[p, i*cols + j] = mat[i*128 + p, j] (k-tile-major free dim)."""
    rows, cols = mat.shape
    kt = rows // 128
    return (mat.reshape(kt, 128, cols).transpose(1, 0, 2)
            .reshape(128, kt * cols))


def _prep_weights(Wx, bx, Wu, bu, Wd, bd, Wy, by):
    A = (2.0 * Wx.T).astype(np.float32)          # [NX, NX]
    Bu = (2.0 * Wu.T).astype(np.float32)         # [NU, NX]
    Bd = Wd.T.astype(np.float32)                 # [ND, NX]
    Cy = Wy.T.astype(np.float32)                 # [NX, NY]
    c = (2.0 * bx + 2.0 * bu + bd).astype(np.float32)

    wa = _to_lhsT_tiles(A).astype(NPBF16)
    wu = _to_lhsT_tiles(Bu).astype(NPBF16)
    wd = _to_lhsT_tiles(Bd).astype(NPBF16)
    wy = _to_lhsT_tiles(Cy).astype(NPBF16)
    cr = np.broadcast_to(c.astype(NPBF16), (128, NX)).copy()
    byr = np.broadcast_to((by.astype(np.float32) / 128.0).astype(NPBF16),
                          (128, NY)).copy()
    one = np.ones((128, 128), dtype=NPBF16)
    idn = np.eye(128, dtype=NPBF16)
    return dict(WA=wa, WU=wu, WD=wd, WY=wy, CR=cr, BYR=byr, ONE=one, IDN=idn)


def _prep_seq(M: np.ndarray) -> np.ndarray:
    """[t, BL, F] fp32 -> [t, 128, F] bf16 transposed per step:
    out[t, p, kt*128+m] = M[t, m, kt*128+p]."""
    t, bl, f = M.shape
    kt = f // 128
    out = (M.transpose(0, 2, 1)                 # [t, F, BL]
           .reshape(t, kt, 128, bl)             # [t, kt, p, m]
           .transpose(0, 2, 1, 3)               # [t, p, kt, m]
           .reshape(t, 128, kt * bl))
    return np.ascontiguousarray(out).astype(NPBF16)


_NC_CACHE = {}


def _get_nc(t_steps: int = T):
    if t_steps not in _NC_CACHE:
        _NC_CACHE[t_steps] = build(t_steps)
    return _NC_CACHE[t_steps]


def kernel(x, U, D, Wx, bx, Wu, bu, Wd, bd, Wy, by, **run_kwargs):
    x = np.asarray(x, dtype=np.float32)
    U = np.asarray(U, dtype=np.float32)
    D = np.asarray(D, dtype=np.float32)
    Wx = np.asarray(Wx, dtype=np.float32)
    bx = np.asarray(bx, dtype=np.float32)
    Wu = np.asarray(Wu, dtype=np.float32)
    bu = np.asarray(bu, dtype=np.float32)
    Wd = np.asarray(Wd, dtype=np.float32)
    bd = np.asarray(bd, dtype=np.float32)
    Wy = np.asarray(Wy, dtype=np.float32)
    by = np.asarray(by, dtype=np.float32)

    t_steps = U.shape[0]
    nc = _get_nc(t_steps)

    weights = _prep_weights(Wx, bx, Wu, bu, Wd, bd, Wy, by)

    in_maps = []
    for cix in range(N_CORES):
        sl = slice(cix * BL, (cix + 1) * BL)
        xT0 = _to_lhsT_tiles(x[sl].T.copy()).astype(NPBF16)
        in_maps.append({
            "xT0": xT0,
            "UT": _prep_seq(U[:, sl]),
            "DT": _prep_seq(D[:, sl]),
            **weights,
        })

    res = run_bass_kernel_spmd(nc, in_maps, core_ids=list(range(N_CORES)),
                               **run_kwargs)

    X = np.concatenate([res.results[cix]["XO"] for cix in range(N_CORES)],
                       axis=1)
    Y = np.concatenate([res.results[cix]["YO"] for cix in range(N_CORES)],
                       axis=1)
    reg = np.zeros((), dtype=np.float32)
    if run_kwargs:
        kernel.last_results = res
    return X, Y, reg
